# revision 40
# baseline (speedup 1.0000x reference)
"""Trainium2 Bass kernel for the torch-faithful MultiHeadAttention module.

Math (validated vs the jax reference):
  qkv = x @ W_qkv.T + b_qkv                    # [B, S, 3E]
  qkv.view(B, H, -1, 3*hd)  is a PLAIN reshape, so "head" h is really the
  sequence block s in [128h, 128h+128), and within a head the 2048 rows are
  s' = (s%128)*16 + j with j = f//192; q/k/v are column slices of each
  192-wide block j.
  score = q @ k.T / 8 ; softmax ; context ; out = context' @ W_out.T + b_out

Sharding (8 cores): data-parallel over batch (4 cores per batch element),
head-parallel within the group (4 heads per core).  Each core computes its
heads' attention entirely on-chip (flash style, no HBM score matrix) and a
partial out-projection over its 256 context columns; the host sums the 4
partials per batch element (a pure unshard/reduce step) and adds b_out.

Internally each head uses the s'' = j*128 + r ordering (a permutation of
s'); the permutation is undone for free in the final strided DMA to DRAM.
"""

import numpy as np

import concourse.bass as bass
import concourse.mybir as mybir
import concourse.tile as tile
from concourse import bacc
from concourse.bass_utils import run_bass_kernel_spmd
from concourse.masks import make_identity

B, S, E = 2, 2048, 1024
H, HD = 16, 64
NH = 4  # heads per core
NJ = 16  # 192-wide column blocks in 3E
P = 128
ET = E // P  # 8 contraction tiles of 128
F32 = mybir.dt.float32
F32R = mybir.dt.float32r
BF16 = mybir.dt.bfloat16
EXP = mybir.ActivationFunctionType.Exp

_NC_CACHE = None
_LAST_RESULT = None  # BassKernelResults of the most recent run (for test harness)


def _emit(nc, tc, xT, wqkvT, woutT, bblk, outp):
    import contextlib

    with contextlib.ExitStack() as ctx:
        ctx.enter_context(
            nc.allow_low_precision(reason="bf16 matmul operands")
        )
        const = ctx.enter_context(tc.tile_pool(name="const", bufs=1))
        vtmp = ctx.enter_context(tc.tile_pool(name="vtmp", bufs=3))
        ppool = ctx.enter_context(tc.tile_pool(name="probs", bufs=6))
        rpool = ctx.enter_context(tc.tile_pool(name="recip", bufs=3))
        # PSUM budget: "w" 2 banks x 2 bufs + "nw" 1 bank x 2 + "ctx" 2 banks
        pwork = ctx.enter_context(tc.tile_pool(name="pwork", bufs=2, space="PSUM"))
        pnorm = ctx.enter_context(tc.tile_pool(name="pnorm", bufs=2, space="PSUM"))
        pctx = ctx.enter_context(tc.tile_pool(name="pctx", bufs=1, space="PSUM"))

        # ---- resident tiles -------------------------------------------------
        xT_sb = const.tile([P, ET, NH * P], BF16, tag="xT")  # [128, 8, 512]
        for et in range(ET):
            nc.sync.dma_start(out=xT_sb[:, et, :], in_=xT[et, :, :])

        # W_qkv^T fully resident in bf16: [128, 8, 3072] = 48KB/partition.
        # Split each row in half so the projection can start after the first
        # half of the columns has landed.
        wq_all = const.tile([P, ET, 3 * E], BF16, tag="wq")
        for qtr in range(4):
            eng = nc.sync if qtr < 2 else nc.gpsimd
            for et in range(ET):
                eng.dma_start(
                    out=wq_all[:, et, qtr * 768:(qtr + 1) * 768],
                    in_=wqkvT[et, :, qtr * 768:(qtr + 1) * 768],
                )

        bblk_sb = const.tile([P, 24], F32, tag="bblk")
        nc.sync.dma_start(out=bblk_sb, in_=bblk[:, :])

        ident = const.tile([P, P], BF16, tag="ident")
        make_identity(nc, ident)
        ident32 = const.tile([P, P], F32, tag="ident32")
        make_identity(nc, ident32)

        # qT/kT per head, s''-ordered columns (separate tensors: matmul
        # operands must share a base partition)
        qT = const.tile([HD, NH, S], BF16, tag="qT")
        kT = const.tile([HD, NH, S], BF16, tag="kT")
        # v_aug per head per j-block: [128 rows, 64 v cols + 1 ones col]
        vaug = const.tile([P, NH, NJ, HD + 1], BF16, tag="vaug")
        # normalized context^T: K-tile t holds heads (2t, 2t+1) on partition halves
        ctxT = const.tile([P, 2, S], BF16, tag="ctxT")
        # the ones column of v_aug is static; set it once
        nc.vector.memset(vaug[:, :, :, HD:HD + 1], 1.0)

        woutT_sb = const.tile([P, 2, E], BF16, tag="woutT")  # [128, 2, 1024]

        qT4 = qT.rearrange("d nh (nj p) -> d nh nj p", p=P)
        kT4 = kT.rearrange("d nh (nj p) -> d nh nj p", p=P)

        # ---- qkv projection (all 4 heads) ----------------------------------
        # One matmul group per 128-wide column block of W_qkv^T (24 blocks).
        # Block pattern per two j:
        #   b=3m: [q_2m | k_2m]  b=3m+1: [v_2m | q_2m+1]  b=3m+2: [k_2m+1 | v_2m+1]
        # Biases come aligned from bblk[p, b] = b_qkv[128 b + p].
        # The projection is interleaved with the first flash chunk: blocks
        # 12-23 are emitted between its kt-subranges so the PE works through
        # the projection while the ACT runs that chunk's exps.
        qT4 = qT.rearrange("d nh (nj p) -> d nh nj p", p=P)
        kT4 = kT.rearrange("d nh (nj p) -> d nh nj p", p=P)
        vts = {}

        def proj_copy(ps, rows, bcol, dest, engine="dve"):
            src_ap = ps[rows[0]:rows[1], :]
            if dest[0] == "qk":
                out_ap = (qT4 if dest[1] == "q" else kT4)[:, :, dest[2], :]
                src_ap = src_ap.rearrange("d (nh p) -> d nh p", p=P)
            else:
                out_ap = vts[dest[2]][:, :]
            bias = bblk_sb[rows[0]:rows[1], bcol:bcol + 1]
            if engine == "act":
                nc.scalar.activation(
                    out=out_ap, in_=src_ap,
                    func=mybir.ActivationFunctionType.Identity, bias=bias,
                )
            else:
                nc.vector.tensor_scalar_add(out=out_ap, in0=src_ap, scalar1=bias)

        def finish_v(j):
            vt_j = vts[j]
            ps_tr = pnorm.tile([P, NH, HD], BF16, tag="nw")
            for h in range(NH):
                nc.tensor.transpose(
                    ps_tr[:, h, :], vt_j[:, h * P:(h + 1) * P],
                    ident[0:HD, 0:HD],
                )
            nc.vector.tensor_copy(out=vaug[:, :, j, 0:HD], in_=ps_tr)

        def proj_block(b):
            ps_b = pwork.tile([P, NH * P], F32, tag="w")
            for et in range(ET):
                nc.tensor.matmul(
                    ps_b,
                    lhsT=wq_all[:, et, b * P:(b + 1) * P],
                    rhs=xT_sb[:, et, :],
                    start=(et == 0),
                    stop=(et == ET - 1),
                )
            m, r = divmod(b, 3)
            if r == 0:
                proj_copy(ps_b, (0, HD), b, ("qk", "q", 2 * m), "act")
                proj_copy(ps_b, (HD, P), b, ("qk", "k", 2 * m))
            elif r == 1:
                vt_new = vtmp.tile([HD, NH * P], BF16, tag="vt")
                vts[2 * m] = vt_new
                proj_copy(ps_b, (0, HD), b, ("v", None, 2 * m), "act")
                proj_copy(ps_b, (HD, P), b, ("qk", "q", 2 * m + 1), "act")
                finish_v(2 * m)
            else:
                vt_new = vtmp.tile([HD, NH * P], BF16, tag="vt")
                vts[2 * m + 1] = vt_new
                proj_copy(ps_b, (0, HD), b, ("qk", "k", 2 * m + 1))
                proj_copy(ps_b, (HD, P), b, ("v", None, 2 * m + 1), "act")
                finish_v(2 * m + 1)

        # ---- flash attention: per head, two 1024-wide query chunks ---------
        # kt-loop software-pipelined (scores two iterations ahead); PSUM
        # context eagerly extracted to SBUF at chunk end; the rest of the
        # normalization deferred until the next chunk's scores are queued.
        CH = 1024
        rscratch = nc.dram_tensor("rinv_scratch", [NH, 2, CH], F32).ap()

        def emit_norm(h, c, l_sb, ctx_sb):
            ps_lt = pnorm.tile([P, 8], F32, tag="nw")
            for kq in range(8):
                nc.tensor.transpose(
                    ps_lt[:, kq:kq + 1], l_sb[0:1, kq * P:(kq + 1) * P],
                    ident32[0:1, 0:1],
                )
            rinv = rpool.tile([P, 8], F32, tag="rinv")
            nc.vector.reciprocal(out=rinv, in_=ps_lt)
            rrow = rpool.tile([1, CH], F32, tag="rrow")
            for half in range(2):
                prow = pnorm.tile([1, 512], F32, tag="nw")
                for q in range(4):
                    kq = half * 4 + q
                    nc.tensor.transpose(
                        prow[0:1, q * P:(q + 1) * P], rinv[:, kq:kq + 1],
                        ident32,
                    )
                nc.vector.tensor_copy(
                    out=rrow[:, half * 512:(half + 1) * 512], in_=prow
                )
            nc.sync.dma_start(out=rscratch[h, c, :], in_=rrow)
            rb = rpool.tile([HD, CH], BF16, tag="rbc")
            nc.gpsimd.dma_start(
                out=rb, in_=rscratch[h, c:c + 1, :].to_broadcast([HD, CH])
            )
            phalf = (h % 2) * HD
            nc.vector.tensor_tensor(
                out=ctxT[phalf:phalf + HD, h // 2, c * CH:(c + 1) * CH],
                in0=ctx_sb,
                in1=rb,
                op=mybir.AluOpType.mult,
            )

        pending = []

        class Chunk:
            def __init__(self, h, c):
                self.h, self.c = h, c
                self.ps_ctx = pctx.tile([HD + 1, CH], F32, tag="ctx")
                self.pTs = [self.scores(0), self.scores(1)]
                if pending:
                    emit_norm(*pending.pop(0))

            def scores(self, kt):
                h, c = self.h, self.c
                pT = ppool.tile([P, CH], BF16, tag="pT")
                ps_s = pwork.tile([P, CH], F32, tag="w")
                for cc in range(2):
                    nc.tensor.matmul(
                        ps_s[:, cc * 512:(cc + 1) * 512],
                        lhsT=kT[:, h, kt * P:(kt + 1) * P],
                        rhs=qT[:, h, c * CH + cc * 512:c * CH + (cc + 1) * 512],
                        start=True,
                        stop=True,
                    )
                # p = exp(score / 8); softmax max-subtraction skipped
                # (scores are O(1) for this problem; validated vs ref)
                nc.scalar.activation(out=pT, in_=ps_s, func=EXP, scale=0.125)
                return pT

            def run(self, kt_lo, kt_hi):
                for kt in range(kt_lo, kt_hi):
                    if kt + 2 < NJ:
                        self.pTs.append(self.scores(kt + 2))
                    cur = self.pTs.pop(0)
                    for cc in range(2):
                        nc.tensor.matmul(
                            self.ps_ctx[:, cc * 512:(cc + 1) * 512],
                            lhsT=vaug[:, self.h, kt, :],
                            rhs=cur[:, cc * 512:(cc + 1) * 512],
                            start=(kt == 0),
                            stop=(kt == NJ - 1),
                        )

            def finish(self):
                l_sb = rpool.tile([1, CH], F32, tag="lrow")
                nc.vector.tensor_copy(out=l_sb, in_=self.ps_ctx[HD:HD + 1, :])
                ctx_sb = rpool.tile([HD, CH], BF16, tag="csb")
                nc.vector.tensor_copy(out=ctx_sb, in_=self.ps_ctx[0:HD, :])
                pending.append((self.h, self.c, l_sb, ctx_sb))

        for b in range(24):
            proj_block(b)

        for t in range(2):
            nc.sync.dma_start(out=woutT_sb[:, t, :], in_=woutT[t, :, :])

        for h in range(NH):
            for c in range(2):
                chk = Chunk(h, c)
                chk.run(0, NJ)
                chk.finish()
        while pending:
            emit_norm(*pending.pop(0))

        # ---- partial out-projection ----------------------------------------
        # out_part[s', f] = sum_{d'} ctxT[d', s''] * woutT[d', f],
        # written to DRAM with the s'' -> s' = 16r + j permutation in the AP.
        out_view = outp.rearrange("(r six) f -> six r f", six=NJ)  # [16, 128, 1024]
        for st in range(NJ):
            o_sb = vtmp.tile([P, E], F32, tag="osb")
            for fc in range(2):
                ps_o = pwork.tile([P, 512], F32, tag="w")
                for ktile in range(2):
                    nc.tensor.matmul(
                        ps_o,
                        lhsT=ctxT[:, ktile, st * P:(st + 1) * P],
                        rhs=woutT_sb[:, ktile, fc * 512:(fc + 1) * 512],
                        start=(ktile == 0),
                        stop=(ktile == 1),
                    )
                if fc == 0:
                    nc.scalar.copy(out=o_sb[:, 0:512], in_=ps_o)
                else:
                    nc.vector.tensor_copy(out=o_sb[:, 512:1024], in_=ps_o)
            nc.sync.dma_start(out=out_view[st, :, :], in_=o_sb)


def build_nc():
    nc = bacc.Bacc("TRN2", target_bir_lowering=False, debug=False, num_devices=8)
    xT = nc.declare_dram_parameter("xT", [ET, P, NH * P], BF16, isOutput=False)
    wqkvT = nc.declare_dram_parameter("wqkvT", [ET, P, 3 * E], BF16, isOutput=False)
    woutT = nc.declare_dram_parameter("woutT", [2, P, E], BF16, isOutput=False)
    bblk = nc.declare_dram_parameter("bblk", [P, 24], F32, isOutput=False)
    outp = nc.declare_dram_parameter("out_part", [S, E], F32, isOutput=True)
    with tile.TileContext(nc) as tc:
        _emit(nc, tc, xT, wqkvT, woutT, bblk, outp)
    nc.compile()
    return nc


def make_in_maps(x, W_qkv, b_qkv, W_out):
    import ml_dtypes
    bf16 = ml_dtypes.bfloat16
    x = np.asarray(x, np.float32)
    # [ET, P, 3E]: wqkvT[et, p, f] = W_qkv.T[et*128+p, f], cast to bf16
    wqkvT = np.ascontiguousarray(
        np.asarray(W_qkv, np.float32).T.reshape(ET, P, 3 * E)
    ).astype(bf16)
    woutT = np.ascontiguousarray(np.asarray(W_out, np.float32).T)
    b_qkv = np.asarray(b_qkv, np.float32)
    bblk = np.ascontiguousarray(np.asarray(b_qkv, np.float32).reshape(24, P).T)
    in_maps = []
    for core in range(8):
        b, g = divmod(core, 4)
        in_maps.append({
            "xT": np.ascontiguousarray(
                x[b, 512 * g:512 * (g + 1), :].T.reshape(ET, P, NH * P)
            ).astype(bf16),
            "wqkvT": wqkvT,
            "woutT": np.ascontiguousarray(
                woutT[256 * g:256 * (g + 1), :].reshape(2, P, E)
            ).astype(bf16),
            "bblk": bblk,
        })
    return in_maps


def kernel(x, W_qkv, b_qkv, W_out, b_out):
    global _NC_CACHE, _LAST_RESULT
    if _NC_CACHE is None:
        _NC_CACHE = build_nc()
    in_maps = make_in_maps(x, W_qkv, b_qkv, W_out)
    _LAST_RESULT = run_bass_kernel_spmd(_NC_CACHE, in_maps, list(range(8)))
    res = _LAST_RESULT.results
    b_out = np.asarray(b_out, np.float32)
    out = np.empty((B, S, E), np.float32)
    for b in range(B):
        acc = np.asarray(res[4 * b]["out_part"], np.float32).copy()
        for g in range(1, 4):
            acc += np.asarray(res[4 * b + g]["out_part"], np.float32)
        out[b] = acc + b_out
    return out
